# revision 6
# baseline (speedup 1.0000x reference)
"""3x3 valid cross-correlation of 64 1024x1024 f32 images on 8 TRN2 NeuronCores.

Strategy: pure data-parallel over batch (8 images per core). On each core an
image is processed as 9 row-blocks (8 full blocks of 128 input rows -> 126
output rows, plus a 16-row tail). The 2D conv runs on the TensorEngine as 3
PSUM-accumulated matmuls per 512-wide column segment: a banded [128, 126]
stationary matrix applies the 3 vertical taps of kernel column dj, while the
moving operand is the image block column-shifted by dj (a free-dim AP offset).

The problem is memory-bound (8.2 MB of f32 HBM traffic per image vs ~12us of
TensorE work), so all device-side I/O is bf16: the host quantizes the input
images and band matrices to bf16, the PSUM->SBUF drain copies cast f32->bf16,
and the host upcasts the bf16 output. This halves HBM traffic (~4.1 MB/image)
for ~4e-3 L2 relative error (tolerance is 2e-2). Input DMAs ride the SP HWDGE
ring, output DMAs the ACT ring, so loads and stores interleave.
"""

import numpy as np

import bass_rust
import concourse.bacc as bacc
import concourse.mybir as mybir
from concourse.tile import TileContext

B = 64          # batch
D = 1024        # image side
O = D - 2      # 1022 output side
N_CORES = 8
BPC = B // N_CORES  # images per core
BLK = 126       # output rows per full block
NBLK = 9        # 8 full blocks + 1 tail
TAIL_M = O - 8 * BLK   # 14 tail output rows
TAIL_K = 16     # tail input rows (1008..1023)

_F32 = mybir.dt.float32
_BF16 = mybir.dt.bfloat16


def _make_bands(ker):
    """Banded stationary matrices from the 3x3 kernel.

    A[k, dj, m] = ker[k-m, dj]  (k-m in 0..2) -> 126 output rows per block
    T[k, dj, m] = ker[k-m, dj] on 16 partitions -> 14 tail output rows
    """
    A = np.zeros((128, 3, BLK), np.float32)
    T = np.zeros((TAIL_K, 3, TAIL_M), np.float32)
    for dj in range(3):
        for di in range(3):
            A[np.arange(BLK) + di, dj, np.arange(BLK)] = ker[di, dj]
            T[np.arange(TAIL_M) + di, dj, np.arange(TAIL_M)] = ker[di, dj]
    return A, T


def _overlap_in_ap(x, img):
    """DRAM AP reading blocks 0..7 of image `img` as [128p, 8b, 1024c] with
    2-row overlap between consecutive blocks (row = 126*b + p)."""
    ap = x.ap()[img]
    c = ap.copy()
    c.ap = bass_rust.VecI64Pair([(D, 128), (BLK * D, 8), (1, D)])
    return c


def _build(loop_iters=None):
    """Build the per-core Bass program. loop_iters wraps the whole workload
    in a For_i loop (benchmarking variant; kernel() uses loop_iters=None)."""
    nc = bacc.Bacc()
    x = nc.dram_tensor("x", [BPC, D, D], _BF16, kind="ExternalInput")
    bandA = nc.dram_tensor("bandA", [128, 3, BLK], _BF16, kind="ExternalInput")
    bandT = nc.dram_tensor("bandT", [TAIL_K, 3, TAIL_M], _BF16, kind="ExternalInput")
    y = nc.dram_tensor("y", [BPC, O, O], _BF16, kind="ExternalOutput")

    with TileContext(nc) as tc:
        with (
            tc.tile_pool(name="bands", bufs=1) as bands,
            tc.tile_pool(name="xin", bufs=3) as xin,
            tc.tile_pool(name="xtail", bufs=3) as xtail,
            tc.tile_pool(name="ps", bufs=4, space="PSUM") as ps,
            tc.tile_pool(name="yout", bufs=3) as yout,
        ):
            A = bands.tile([128, 3, BLK], _BF16)
            T = bands.tile([TAIL_K, 3, TAIL_M], _BF16)
            nc.sync.dma_start(A[:], bandA[:])
            nc.sync.dma_start(T[:], bandT[:])

            def one_image(img):
                X = xin.tile([128, 8, D], _BF16, tag="x")
                XT = xtail.tile([TAIL_K, D], _BF16, tag="xt")
                nc.sync.dma_start(X[:], _overlap_in_ap(x, img))
                nc.sync.dma_start(XT[:], x[img, D - TAIL_K : D, :])

                Y = yout.tile([128, NBLK, O], _BF16, tag="y")
                for b in range(NBLK):
                    tail = b == NBLK - 1
                    m = TAIL_M if tail else BLK
                    W = T if tail else A
                    P = ps.tile([128, O], _F32, tag="p")
                    for s0, sl in ((0, 512), (512, 510)):
                        for dj in range(3):
                            nc.tensor.matmul(
                                P[:m, s0 : s0 + sl],
                                lhsT=W[:, dj, :m],
                                rhs=(XT if tail else X[:, b])[
                                    :, dj + s0 : dj + s0 + sl
                                ],
                                start=(dj == 0),
                                stop=(dj == 2),
                            )
                    if b % 2 == 0:
                        nc.scalar.copy(Y[:m, b, :], P[:m, :])
                    else:
                        nc.vector.tensor_copy(Y[:m, b, :], P[:m, :])

                # stores via SWDGE on the otherwise-idle Pool engine, so the
                # ACT engine only does PSUM drains
                nc.gpsimd.dma_start(
                    y[img, 0 : 8 * BLK, :].rearrange("(b p) c -> p b c", p=BLK),
                    Y[:BLK, 0:8, :],
                )
                nc.gpsimd.dma_start(y[img, 8 * BLK : O, :], Y[:TAIL_M, 8, :])

            def all_images():
                for img in range(BPC):
                    one_image(img)

            if loop_iters is None:
                all_images()
            else:
                with tc.For_i(0, loop_iters, 1):
                    all_images()
    nc.compile()
    return nc


_CACHE = {}


def _make_runner(nc, donate=True):
    """Wrap a finalized Bass program in a jitted SPMD runner.

    Mirrors run_bass_via_pjrt: operands are (inputs..., zero outputs...,
    partition-id), in exactly the jit parameter order neuronx_cc_hook
    requires.
    """
    import jax
    from jax.sharding import Mesh, PartitionSpec
    from jax.experimental.shard_map import shard_map
    from concourse.bass2jax import (
        _bass_exec_p,
        partition_id_tensor,
        install_neuronx_cc_hook,
    )

    install_neuronx_cc_hook()
    partition_name = nc.partition_id_tensor.name if nc.partition_id_tensor else None

    in_names, out_names, out_avals, zero_outs = [], [], [], []
    for alloc in nc.m.functions[0].allocations:
        if not isinstance(alloc, mybir.MemoryLocationSet):
            continue
        name = alloc.memorylocations[0].name
        if alloc.kind == "ExternalInput":
            if name != partition_name:
                in_names.append(name)
        elif alloc.kind == "ExternalOutput":
            out_names.append(name)
            shape = tuple(alloc.tensor_shape)
            dtype = mybir.dt.np(alloc.dtype)
            out_avals.append(jax.core.ShapedArray(shape, dtype))
            zero_outs.append(np.zeros(shape, dtype))
    n_params = len(in_names)
    n_outs = len(out_avals)
    all_names = in_names + out_names
    if partition_name is not None:
        all_names.append(partition_name)

    def _body(*args):
        outs = _bass_exec_p.bind(
            *args,
            partition_id_tensor(),
            out_avals=tuple(out_avals),
            in_names=tuple(all_names),
            out_names=tuple(out_names),
            lowering_input_output_aliases=(),
            sim_require_finite=True,
            sim_require_nnan=True,
            nc=nc,
        )
        return tuple(outs)

    devices = jax.devices()[:N_CORES]
    mesh = Mesh(np.asarray(devices), ("core",))
    fn = jax.jit(
        shard_map(
            _body,
            mesh=mesh,
            in_specs=(PartitionSpec("core"),) * (n_params + n_outs),
            out_specs=(PartitionSpec("core"),) * n_outs,
            check_rep=False,
        ),
        donate_argnums=(
            tuple(range(n_params, n_params + n_outs)) if donate else ()
        ),
        keep_unused=True,
    )
    return fn, in_names, out_names, zero_outs


def _get_runner(loop_iters=None, donate=True):
    key = ("runner", loop_iters, donate)
    if key not in _CACHE:
        _CACHE[key] = _make_runner(_build(loop_iters), donate=donate)
    return _CACHE[key]


def _concat_inputs(inputs, ker):
    import ml_dtypes

    bf16 = np.dtype(ml_dtypes.bfloat16)
    A, T = _make_bands(np.asarray(ker, np.float32).reshape(3, 3))
    x = np.ascontiguousarray(
        np.asarray(inputs, np.float32).astype(bf16)
    ).reshape(B, D, D)
    return {
        "x": x,
        "bandA": np.ascontiguousarray(
            np.broadcast_to(A.astype(bf16), (N_CORES,) + A.shape)
        ).reshape(N_CORES * 128, 3, BLK),
        "bandT": np.ascontiguousarray(
            np.broadcast_to(T.astype(bf16), (N_CORES,) + T.shape)
        ).reshape(N_CORES * TAIL_K, 3, TAIL_M),
    }


def kernel(inputs, kernel):
    import jax

    fn, in_names, out_names, zero_outs = _get_runner()
    concat = _concat_inputs(inputs, kernel)
    zeros = [
        np.zeros((N_CORES * z.shape[0], *z.shape[1:]), z.dtype) for z in zero_outs
    ]
    out = fn(*[concat[n] for n in in_names], *zeros)
    y = np.asarray(jax.block_until_ready(out)[0])
    return y.astype(np.float32).reshape(B, O * O)



# revision 9
# speedup vs baseline: 2.9958x; 2.9958x over previous
"""3x3 valid cross-correlation of 64 1024x1024 f32 images on 8 TRN2 NeuronCores.

Strategy: pure data-parallel over batch (8 images per core). On each core an
image is processed as 9 row-blocks (8 full blocks of 128 input rows -> 126
output rows, plus a 16-row tail). The 2D conv runs on the TensorEngine as 3
PSUM-accumulated matmuls per 512-wide column segment: a banded [128, 126]
stationary matrix applies the 3 vertical taps of kernel column dj, while the
moving operand is the image block column-shifted by dj (a free-dim AP offset).

The problem is memory-bound, so all device-side I/O is bf16 (host quantizes
input, host upcasts output; ~3e-3 L2 error vs the 2e-2 gate) and both input
and output use host-side pre/post-permuted layouts in which every DMA reads
or writes a single contiguous 8 KiB run per partition: xp[i,p,b,:] holds
image row 126*b+p, yp[i,p,b,:] holds output row 126*b+p. A DMA probe on this
hardware measured 392 GB/s single-core / 324 GB/s per-core with all 8 cores
streaming this layout, vs ~140 GB/s effective for the naive row-strided APs
(2 KiB scattered descriptors). Loads ride the SP HWDGE ring, stores the ACT
ring; each is split in halves so the PE starts after half a load and stores
drain smoothly. PSUM is drained to SBUF alternately by the ACT and DVE
engines, casting f32->bf16.
"""

import numpy as np

import bass_rust
import concourse.bacc as bacc
import concourse.mybir as mybir
from concourse.tile import TileContext

B = 64          # batch
D = 1024        # image side
O = D - 2       # 1022 output side
N_CORES = 8
BPC = B // N_CORES  # images per core
BLK = 126       # output rows per full block
NBLK = 9        # 8 full blocks + 1 tail
TAIL_M = O - 8 * BLK   # 14 tail output rows
TAIL_K = 16     # tail input rows (1008..1023)

_F32 = mybir.dt.float32
_BF16 = mybir.dt.bfloat16


def _make_bands(ker):
    """Banded stationary matrices from the 3x3 kernel.

    A[k, dj, m] = ker[k-m, dj]  (k-m in 0..2) -> 126 output rows per block
    T[k, dj, m] = ker[k-m, dj] on 16 partitions -> 14 tail output rows
    """
    A = np.zeros((128, 3, BLK), np.float32)
    T = np.zeros((TAIL_K, 3, TAIL_M), np.float32)
    for dj in range(3):
        for di in range(3):
            A[np.arange(BLK) + di, dj, np.arange(BLK)] = ker[di, dj]
            T[np.arange(TAIL_M) + di, dj, np.arange(TAIL_M)] = ker[di, dj]
    return A, T


def _build(loop_iters=None):
    """Build the per-core Bass program. loop_iters wraps the whole workload
    in a For_i loop (benchmarking variant; kernel() uses loop_iters=None)."""
    nc = bacc.Bacc()
    # xp[i, p, b, :] = image row 126*b + p  (contiguous 8 KiB per partition
    # per half-load); xt[i] = rows 1008..1023 for the tail block.
    xp = nc.dram_tensor("xp", [BPC, 128, 8, D], _BF16, kind="ExternalInput")
    xt = nc.dram_tensor("xt", [BPC, TAIL_K, D], _BF16, kind="ExternalInput")
    bandA = nc.dram_tensor("bandA", [128, 3, BLK], _BF16, kind="ExternalInput")
    bandT = nc.dram_tensor("bandT", [TAIL_K, 3, TAIL_M], _BF16, kind="ExternalInput")
    # yp[i, p, b, :] = output row 126*b + p (b in 0..7); yt[i] = rows
    # 1008..1021. The host unpermutes.
    yp = nc.dram_tensor("yp", [BPC, BLK, 8, O], _BF16, kind="ExternalOutput")
    yt = nc.dram_tensor("yt", [BPC, TAIL_M, O], _BF16, kind="ExternalOutput")

    with TileContext(nc) as tc:
        with (
            tc.tile_pool(name="bands", bufs=1) as bands,
            tc.tile_pool(name="xin", bufs=3) as xin,
            tc.tile_pool(name="xtail", bufs=3) as xtail,
            tc.tile_pool(name="ps", bufs=4, space="PSUM") as ps,
            tc.tile_pool(name="yout", bufs=3) as yout,
            tc.tile_pool(name="ytail", bufs=3) as ytail,
        ):
            A = bands.tile([128, 3, BLK], _BF16)
            T = bands.tile([TAIL_K, 3, TAIL_M], _BF16)
            nc.sync.dma_start(A[:], bandA[:])
            nc.sync.dma_start(T[:], bandT[:])

            def one_image(img):
                X0 = xin.tile([128, 4, D], _BF16, tag="x0")
                X1 = xin.tile([128, 4, D], _BF16, tag="x1")
                XT = xtail.tile([TAIL_K, D], _BF16, tag="xt")
                nc.sync.dma_start(X0[:], xp[img, :, 0:4, :])
                nc.sync.dma_start(X1[:], xp[img, :, 4:8, :])
                nc.sync.dma_start(XT[:], xt[img])

                Y0 = yout.tile([BLK, 4, O], _BF16, tag="y0")
                Y1 = yout.tile([BLK, 4, O], _BF16, tag="y1")
                YT = ytail.tile([TAIL_M, O], _BF16, tag="yt")
                for b in range(NBLK):
                    tail = b == NBLK - 1
                    m = TAIL_M if tail else BLK
                    W = T if tail else A
                    X = XT if tail else (X0[:, b] if b < 4 else X1[:, b - 4])
                    P = ps.tile([128, O], _F32, tag="p")
                    for s0, sl in ((0, 512), (512, 510)):
                        for dj in range(3):
                            nc.tensor.matmul(
                                P[:m, s0 : s0 + sl],
                                lhsT=W[:, dj, :m],
                                rhs=X[:, dj + s0 : dj + s0 + sl],
                                start=(dj == 0),
                                stop=(dj == 2),
                            )
                    dst = YT[:] if tail else (Y0 if b < 4 else Y1)[:, b % 4, :]
                    if b % 2 == 0 and not tail:
                        nc.scalar.copy(dst[:m], P[:m, :])
                    else:
                        nc.vector.tensor_copy(dst[:m], P[:m, :])
                    if b == 3:
                        nc.scalar.dma_start(yp[img, :, 0:4, :], Y0[:])
                # stores on the ACT HWDGE ring (separate FIFO from loads)
                nc.scalar.dma_start(yp[img, :, 4:8, :], Y1[:])
                nc.scalar.dma_start(yt[img], YT[:])

            def all_images():
                for img in range(BPC):
                    one_image(img)

            if loop_iters is None:
                all_images()
            else:
                with tc.For_i(0, loop_iters, 1):
                    all_images()
    nc.compile()
    return nc


_CACHE = {}


def _make_runner(nc, donate=True):
    """Wrap a finalized Bass program in a jitted SPMD runner.

    Mirrors run_bass_via_pjrt: operands are (inputs..., zero outputs...,
    partition-id), in exactly the jit parameter order neuronx_cc_hook
    requires.
    """
    import jax
    from jax.sharding import Mesh, PartitionSpec
    from jax.experimental.shard_map import shard_map
    from concourse.bass2jax import (
        _bass_exec_p,
        partition_id_tensor,
        install_neuronx_cc_hook,
    )

    install_neuronx_cc_hook()
    partition_name = nc.partition_id_tensor.name if nc.partition_id_tensor else None

    in_names, out_names, out_avals, zero_outs = [], [], [], []
    for alloc in nc.m.functions[0].allocations:
        if not isinstance(alloc, mybir.MemoryLocationSet):
            continue
        name = alloc.memorylocations[0].name
        if alloc.kind == "ExternalInput":
            if name != partition_name:
                in_names.append(name)
        elif alloc.kind == "ExternalOutput":
            out_names.append(name)
            shape = tuple(alloc.tensor_shape)
            dtype = mybir.dt.np(alloc.dtype)
            out_avals.append(jax.core.ShapedArray(shape, dtype))
            zero_outs.append(np.zeros(shape, dtype))
    n_params = len(in_names)
    n_outs = len(out_avals)
    all_names = in_names + out_names
    if partition_name is not None:
        all_names.append(partition_name)

    def _body(*args):
        outs = _bass_exec_p.bind(
            *args,
            partition_id_tensor(),
            out_avals=tuple(out_avals),
            in_names=tuple(all_names),
            out_names=tuple(out_names),
            lowering_input_output_aliases=(),
            sim_require_finite=True,
            sim_require_nnan=True,
            nc=nc,
        )
        return tuple(outs)

    devices = jax.devices()[:N_CORES]
    mesh = Mesh(np.asarray(devices), ("core",))
    fn = jax.jit(
        shard_map(
            _body,
            mesh=mesh,
            in_specs=(PartitionSpec("core"),) * (n_params + n_outs),
            out_specs=(PartitionSpec("core"),) * n_outs,
            check_rep=False,
        ),
        donate_argnums=(
            tuple(range(n_params, n_params + n_outs)) if donate else ()
        ),
        keep_unused=True,
    )
    return fn, in_names, out_names, zero_outs


def _get_runner(loop_iters=None, donate=True):
    key = ("runner", loop_iters, donate)
    if key not in _CACHE:
        _CACHE[key] = _make_runner(_build(loop_iters), donate=donate)
    return _CACHE[key]


def _concat_inputs(inputs, ker):
    import ml_dtypes

    bf16 = np.dtype(ml_dtypes.bfloat16)
    A, T = _make_bands(np.asarray(ker, np.float32).reshape(3, 3))
    x = np.asarray(inputs, np.float32).astype(bf16).reshape(B, D, D)
    # xp[i, p, b, :] = x[i, 126*b + p, :] — duplicates the 2-row overlap
    # between consecutive blocks so each partition's rows are contiguous.
    si, sr, sc = x.strides
    xp = np.lib.stride_tricks.as_strided(
        x, shape=(B, 128, 8, D), strides=(si, sr, BLK * sr, sc)
    )
    return {
        "xp": np.ascontiguousarray(xp),
        "xt": np.ascontiguousarray(x[:, D - TAIL_K :, :]),
        "bandA": np.ascontiguousarray(
            np.broadcast_to(A.astype(bf16), (N_CORES,) + A.shape)
        ).reshape(N_CORES * 128, 3, BLK),
        "bandT": np.ascontiguousarray(
            np.broadcast_to(T.astype(bf16), (N_CORES,) + T.shape)
        ).reshape(N_CORES * TAIL_K, 3, TAIL_M),
    }


def kernel(inputs, kernel):
    import jax

    fn, in_names, out_names, zero_outs = _get_runner()
    concat = _concat_inputs(inputs, kernel)
    zeros = [
        np.zeros((N_CORES * z.shape[0], *z.shape[1:]), z.dtype) for z in zero_outs
    ]
    outs = fn(*[concat[n] for n in in_names], *zeros)
    outs = jax.block_until_ready(outs)
    om = dict(zip(out_names, outs))
    yp = np.asarray(om["yp"]).reshape(B, BLK, 8, O)   # [i, p, b, c]
    yt = np.asarray(om["yt"]).reshape(B, TAIL_M, O)
    y = np.empty((B, O, O), np.float32)
    y[:, : 8 * BLK, :] = (
        yp.transpose(0, 2, 1, 3).astype(np.float32).reshape(B, 8 * BLK, O)
    )
    y[:, 8 * BLK :, :] = yt.astype(np.float32)
    return y.reshape(B, O * O)
